# revision 7
# baseline (speedup 1.0000x reference)
"""Quantized linear (dynamic per-tensor int8) on 8 TRN2 NeuronCores.

Reference semantics:
    x_q = round(x / s_x), s_x = max|x|/127   (per-tensor, round-half-even)
    w_q = round(w / s_w), s_w = max|w|/127
    out = (x_q @ w_q.T) * (s_x * s_w) + bias

Distribution: data-parallel over M (8 shards of 1024 rows), weight
replicated.  Each core scans a disjoint 1/8 of x (its own shard) and of w
for the local absmax; a single 2-element AllReduce(max) collective produces
both global scales in one round trip.  Quantized values are exact small
integers held in fp16 (ints <= 2047 are exact in fp16), so the TensorE fp16
matmul with fp32 PSUM accumulation reproduces the int8 GEMM exactly (all
partial sums stay far below 2^24).

Rounding uses an fp16 magic: (v*inv_s + 1536) written to fp16 rounds the
fractional part half-to-even (ulp = 1 in [1024, 2048)), then an in-place
fp16 subtract of 1536 (2x DVE rate) recovers the integer, matching
jnp.round.

Scheduling notes (engine FIFOs execute in emission order):
  * All large f32 staging goes through ONE pool tag ("stg", 10 x 1 MiB
    ring).  The ring's WAR chain defers every re-read/prefetch DMA until
    the absmax-scan chunks it displaces have been consumed, so the scan
    (which gates the collective) gets full HBM bandwidth first, and the
    re-reads then land inside the collective's latency window.
  * One fused AllReduce instead of two staggered ones removes a full
    collective round trip plus the strict-FIFO GpSimd serialization
    between them.
  * The w-strip quantize for strip s+2 is emitted BEFORE the PSUM
    evacuations of strip s so the DVE FIFO never makes the TensorE wait.
"""

import numpy as np

from concourse import bacc, bass_isa
import concourse.bass_utils as bass_utils
import concourse.mybir as mybir
import concourse.tile as tile

P = 128
M, K, N = 8192, 4096, 4096
NCORES = 8
MLOC = M // NCORES  # 1024 rows of x per core
WS = N // NCORES  # 512 columns of wT scanned per core for absmax
MAGIC = 1536.0  # fp16 round-to-int magic: [1024,2048) has ulp 1
MFREE = 512  # moving free dim per matmul (one fp32 PSUM bank)
NSTRIP = 128  # n-columns of w quantized per strip
INV127 = float(np.float32(1.0 / 127.0))

F32 = mybir.dt.float32
F16 = mybir.dt.float16
AX = mybir.AxisListType
ALU = mybir.AluOpType
ACTF = mybir.ActivationFunctionType


def build_body(tc, xT, wT, wscanT, bias, outT, *, n_cores):
    nc = tc.nc
    k, m_loc = xT.shape
    n = wT.shape[1]
    kt_n = k // P  # 32
    n_strips = n // NSTRIP  # 32
    n_mh = m_loc // MFREE  # 2
    n_ck = kt_n // 4  # 8 quantize chunks of 4 k-tiles per mh

    with (
        tc.tile_pool(name="const", bufs=1) as const,
        tc.tile_pool(name="stats", bufs=1) as stats,
        tc.tile_pool(name="stage", bufs=12) as stage,
        tc.tile_pool(name="xq", bufs=1) as xq_pool,
        tc.tile_pool(name="wq", bufs=4) as wq_pool,
        tc.tile_pool(name="ob", bufs=4) as ob_pool,
        tc.tile_pool(name="ps", bufs=6, space="PSUM") as ps_pool,
        tc.tile_pool(name="dram", bufs=1, space="DRAM") as dram,
    ):
        # ---- bias, laid out bias[s*128+p] -> bias_sb[p, s] ---------------
        bias_sb = const.tile([P, n // P], F32)
        nc.sync.dma_start(bias_sb[:], bias.rearrange("(nt p) -> p nt", p=P))

        xT3 = xT.rearrange("(c p) m -> p c m", p=P)  # [128, 32, 1024]
        wsT3 = wscanT.rearrange("(c p) m -> p c m", p=P)  # [128, 32, 512]
        wT3 = wT.rearrange("(kt p) n -> p kt n", p=P)  # [128, 32, 4096]

        # ---- 1. absmax scans (ring allocs 0..23: run at full HBM BW) ----
        wmax_cols = stats.tile([P, 8], F32)
        for i in range(8):
            tw = stage.tile([P, 4, WS], F32, tag="stg", name=f"wsc{i}")
            nc.sync.dma_start(tw[:], wsT3[:, i * 4 : (i + 1) * 4, :])
            nc.vector.tensor_reduce(
                wmax_cols[:, i : i + 1], tw[:], axis=AX.XY, op=ALU.max,
                apply_absolute_value=True,
            )
        xmax_cols = stats.tile([P, 16], F32)
        for i in range(16):
            tx = stage.tile([P, 2, m_loc], F32, tag="stg", name=f"xsc{i}")
            nc.sync.dma_start(tx[:], xT3[:, i * 2 : (i + 1) * 2, :])
            nc.vector.tensor_reduce(
                xmax_cols[:, i : i + 1], tx[:], axis=AX.XY, op=ALU.max,
                apply_absolute_value=True,
            )
        lmax2 = stats.tile([P, 2], F32)
        nc.vector.tensor_reduce(lmax2[:, 0:1], wmax_cols[:], axis=AX.X, op=ALU.max)
        nc.vector.tensor_reduce(lmax2[:, 1:2], xmax_cols[:], axis=AX.X, op=ALU.max)

        # ---- 2. ONE fused collective: AllReduce(max) over [wmax, xmax] --
        gmax2 = stats.tile([P, 2], F32)
        nc.gpsimd.partition_all_reduce(
            gmax2[:, 0:1], lmax2[:, 0:1], channels=P,
            reduce_op=bass_isa.ReduceOp.max,
        )
        nc.gpsimd.partition_all_reduce(
            gmax2[:, 1:2], lmax2[:, 1:2], channels=P,
            reduce_op=bass_isa.ReduceOp.max,
        )
        cc_in = dram.tile([1, 2], F32)
        cc_out = dram.tile([1, 2], F32)
        # Tiny collective DMAs ride the otherwise-idle Activation HWDGE ring
        # so they don't queue behind in-flight multi-MiB prefetch packets.
        nc.scalar.dma_start(cc_in[:], gmax2[0:1, :])
        nc.gpsimd.collective_compute(
            "AllReduce", ALU.max, replica_groups=[list(range(n_cores))],
            ins=[cc_in.opt()], outs=[cc_out.opt()],
        )

        # ---- 3. gated prefetch: ring WAR defers these behind the scans --
        # (DMA triggers only; consumed post-collective.)
        wf_tiles = {}  # (s, half) -> f32 stage tile [P, 16, 128]
        xre_tiles = {}  # (mh, ck) -> f32 stage tile [P, 4, 512]

        def load_wf(s):
            for h in range(2):
                t = stage.tile([P, 16, NSTRIP], F32, tag="stg", name=f"wf{s}_{h}")
                nc.sync.dma_start(
                    t[:],
                    wT3[:, h * 16 : (h + 1) * 16,
                        s * NSTRIP : (s + 1) * NSTRIP],
                )
                wf_tiles[(s, h)] = t

        def load_xre(mh, ck):
            t = stage.tile([P, 4, MFREE], F32, tag="stg", name=f"xr{mh}_{ck}")
            nc.sync.dma_start(
                t[:],
                xT3[:, ck * 4 : (ck + 1) * 4,
                    mh * MFREE : (mh + 1) * MFREE],
            )
            xre_tiles[(mh, ck)] = t

        load_wf(0)
        for ck in range(n_ck):
            load_xre(0, ck)
        load_wf(1)
        for ck in range(n_ck):
            load_xre(1, ck)
        load_wf(2)
        load_wf(3)

        # ---- 4. scales from the collective -------------------------------
        gsb = stats.tile([1, 2], F32)
        nc.scalar.dma_start(gsb[:], cc_out[:])
        wrec = stats.tile([1, 1], F32)
        xrec = stats.tile([1, 1], F32)
        s_w = stats.tile([1, 1], F32)
        s_x = stats.tile([1, 1], F32)
        sc4 = stats.tile([1, 4], F32)
        nc.vector.reciprocal(wrec[:], gsb[:, 0:1])
        nc.vector.reciprocal(xrec[:], gsb[:, 1:2])
        nc.vector.tensor_scalar(sc4[:, 0:1], wrec[:], 127.0, None, op0=ALU.mult)
        nc.vector.tensor_scalar(sc4[:, 1:2], xrec[:], 127.0, None, op0=ALU.mult)
        nc.vector.tensor_scalar(s_w[:], gsb[:, 0:1], INV127, None, op0=ALU.mult)
        nc.vector.tensor_scalar(s_x[:], gsb[:, 1:2], INV127, None, op0=ALU.mult)
        nc.vector.tensor_tensor(sc4[:, 2:3], s_w[:], s_x[:], op=ALU.mult)
        scb = const.tile([P, 4], F32)
        nc.gpsimd.partition_broadcast(scb[:], sc4[:])
        inv_sw = scb[:, 0:1]
        inv_sx = scb[:, 1:2]
        out_sc = scb[:, 2:3]

        # ---- 5. quantize helpers ----------------------------------------
        wq_tiles = {}

        def quant_w_strip(s, on_act=True):
            wq = wq_pool.tile([P, kt_n, NSTRIP], F16, tag="wq", name=f"wq{s}")
            for h in range(2):
                sl = wq[:, h * 16 : (h + 1) * 16, :]
                src = wf_tiles.pop((s, h))[:]
                if on_act:
                    nc.scalar.activation(
                        sl, src, ACTF.Copy, bias=MAGIC, scale=inv_sw
                    )
                else:
                    nc.vector.tensor_scalar(
                        sl, src, inv_sw, MAGIC, op0=ALU.mult, op1=ALU.add
                    )
                nc.vector.tensor_scalar(sl, sl, MAGIC, None, op0=ALU.subtract)
            wq_tiles[s] = wq

        xqs = [
            xq_pool.tile([P, kt_n, MFREE], F16, tag=f"xq{h}", name=f"xq{h}")
            for h in range(n_mh)
        ]

        def quant_x_chunk(mh, ck, on_act):
            sl = xqs[mh][:, ck * 4 : (ck + 1) * 4, :]
            src = xre_tiles.pop((mh, ck))[:]
            if on_act:
                nc.scalar.activation(sl, src, ACTF.Copy, bias=MAGIC, scale=inv_sx)
            else:
                nc.vector.tensor_scalar(
                    sl, src, inv_sx, MAGIC, op0=ALU.mult, op1=ALU.add
                )
            nc.vector.tensor_scalar(sl, sl, MAGIC, None, op0=ALU.subtract)

        # ---- 6. quantize prelude (strips 0-1, all of x) -----------------
        # Strip 0 entirely on DVE: the first matmul fires ~4us after the
        # scales land instead of waiting for a ScalarE activation pass.
        quant_w_strip(0, on_act=False)
        quant_x_chunk(0, 0, on_act=False)
        for ck in range(1, n_ck):
            quant_x_chunk(0, ck, on_act=(ck % 2 == 0))
        quant_w_strip(1)
        for ck in range(n_ck):
            quant_x_chunk(1, ck, on_act=(ck % 2 == 0))

        # ---- 7. stream: per strip s: prefetch s+4, quantize s+2, MM s ---
        for s in range(n_strips):
            if s + 4 < n_strips:
                load_wf(s + 4)
            if s + 2 < n_strips:
                quant_w_strip(s + 2)
            wq = wq_tiles.pop(s)
            for mh in range(n_mh):
                ps = ps_pool.tile([P, MFREE], F32)
                for kt in range(kt_n):
                    nc.tensor.matmul(
                        ps[:],
                        wq[:, kt, :],
                        xqs[mh][:, kt, :],
                        start=(kt == 0),
                        stop=(kt == kt_n - 1),
                    )
                ob = ob_pool.tile([P, MFREE], F32, tag="ob")
                nc.vector.tensor_scalar(
                    ob[:], ps[:], out_sc, bias_sb[:, s : s + 1],
                    op0=ALU.mult, op1=ALU.add,
                )
                nc.gpsimd.dma_start(
                    outT[s * NSTRIP : (s + 1) * NSTRIP,
                         mh * MFREE : (mh + 1) * MFREE],
                    ob[:],
                )


def build_nc(m_loc=MLOC, k=K, n=N, ws=WS, n_cores=NCORES):
    nc = bacc.Bacc("TRN2", target_bir_lowering=False, debug=False,
                   num_devices=n_cores)
    xT = nc.dram_tensor("xT", [k, m_loc], F32, kind="ExternalInput").ap()
    wT = nc.dram_tensor("wT", [k, n], F32, kind="ExternalInput").ap()
    wscanT = nc.dram_tensor("wscanT", [k, ws], F32, kind="ExternalInput").ap()
    bias = nc.dram_tensor("bias", [n], F32, kind="ExternalInput").ap()
    outT = nc.dram_tensor("outT", [n, m_loc], F32, kind="ExternalOutput").ap()
    with tile.TileContext(nc) as tc:
        build_body(tc, xT, wT, wscanT, bias, outT, n_cores=n_cores)
    nc.compile()
    return nc


def make_in_maps(x, weight, bias, n_cores=NCORES):
    m_loc = x.shape[0] // n_cores
    ws = weight.shape[0] // n_cores
    wT = np.ascontiguousarray(weight.T)
    bias = np.ascontiguousarray(bias, dtype=np.float32)
    maps = []
    for c in range(n_cores):
        maps.append({
            "xT": np.ascontiguousarray(x[c * m_loc : (c + 1) * m_loc].T),
            "wT": wT,
            "wscanT": np.ascontiguousarray(weight[c * ws : (c + 1) * ws].T),
            "bias": bias,
        })
    return maps


_NC_CACHE = {}
LAST_RUN = None


def kernel(x, weight, bias, _trace=False):
    global LAST_RUN
    x = np.ascontiguousarray(np.asarray(x), dtype=np.float32)
    weight = np.ascontiguousarray(np.asarray(weight), dtype=np.float32)
    bias = np.asarray(bias, dtype=np.float32)
    if "full" not in _NC_CACHE:
        _NC_CACHE["full"] = build_nc()
    nc = _NC_CACHE["full"]
    in_maps = make_in_maps(x, weight, bias)
    res = bass_utils.run_bass_kernel_spmd(
        nc, in_maps, core_ids=list(range(NCORES)), trace=_trace
    )
    LAST_RUN = res
    out = np.empty((M, N), np.float32)
    for c in range(NCORES):
        out[c * MLOC : (c + 1) * MLOC, :] = res.results[c]["outT"].T
    return out


# revision 11
# speedup vs baseline: 1.0003x; 1.0003x over previous
"""Quantized linear (dynamic per-tensor int8) on 8 TRN2 NeuronCores.

Reference semantics:
    x_q = round(x / s_x), s_x = max|x|/127   (per-tensor, round-half-even)
    w_q = round(w / s_w), s_w = max|w|/127
    out = (x_q @ w_q.T) * (s_x * s_w) + bias

Distribution: data-parallel over M (8 shards of 1024 rows), weight
replicated.  Each core scans a disjoint 1/8 of x (its own shard) and of w
for the local absmax; a single 2-element AllReduce(max) collective produces
both global scales in one round trip.  Quantized values are exact small
integers held in fp16 (ints <= 2047 are exact in fp16), so the TensorE fp16
matmul with fp32 PSUM accumulation reproduces the int8 GEMM exactly (all
partial sums stay far below 2^24).

Rounding uses an fp16 magic: (v*inv_s + 1536) written to fp16 rounds the
fractional part half-to-even (ulp = 1 in [1024, 2048)), then an in-place
fp16 subtract of 1536 (2x DVE rate) recovers the integer, matching
jnp.round.

Scheduling notes (engine FIFOs execute in emission order):
  * All large f32 staging goes through ONE pool tag ("stg", 10 x 1 MiB
    ring).  The ring's WAR chain defers every re-read/prefetch DMA until
    the absmax-scan chunks it displaces have been consumed, so the scan
    (which gates the collective) gets full HBM bandwidth first, and the
    re-reads then land inside the collective's latency window.
  * One fused AllReduce instead of two staggered ones removes a full
    collective round trip plus the strict-FIFO GpSimd serialization
    between them.
  * The w-strip quantize for strip s+2 is emitted BEFORE the PSUM
    evacuations of strip s so the DVE FIFO never makes the TensorE wait.
"""

import numpy as np

from concourse import bacc, bass_isa
import concourse.bass_utils as bass_utils
import concourse.mybir as mybir
import concourse.tile as tile

P = 128
M, K, N = 8192, 4096, 4096
NCORES = 8
MLOC = M // NCORES  # 1024 rows of x per core
WS = N // NCORES  # 512 columns of wT scanned per core for absmax
MAGIC = 1536.0  # fp16 round-to-int magic: [1024,2048) has ulp 1
MFREE = 512  # moving free dim per matmul (one fp32 PSUM bank)
NSTRIP = 128  # n-columns of w quantized per strip
INV127 = float(np.float32(1.0 / 127.0))

F32 = mybir.dt.float32
F16 = mybir.dt.float16
AX = mybir.AxisListType
ALU = mybir.AluOpType
ACTF = mybir.ActivationFunctionType


def build_body(tc, xT, wT, wscanT, bias, outT, *, n_cores):
    nc = tc.nc
    k, m_loc = xT.shape
    n = wT.shape[1]
    kt_n = k // P  # 32
    n_strips = n // NSTRIP  # 32
    n_mh = m_loc // MFREE  # 2
    n_ck = kt_n // 4  # 8 quantize chunks of 4 k-tiles per mh

    with (
        tc.tile_pool(name="const", bufs=1) as const,
        tc.tile_pool(name="stats", bufs=1) as stats,
        tc.tile_pool(name="stage", bufs=12) as stage,
        tc.tile_pool(name="xq", bufs=1) as xq_pool,
        tc.tile_pool(name="wq", bufs=4) as wq_pool,
        tc.tile_pool(name="ob", bufs=4) as ob_pool,
        tc.tile_pool(name="ps", bufs=6, space="PSUM") as ps_pool,
        tc.tile_pool(name="dram", bufs=1, space="DRAM") as dram,
    ):
        # ---- bias, laid out bias[s*128+p] -> bias_sb[p, s] ---------------
        bias_sb = const.tile([P, n // P], F32)
        nc.sync.dma_start(bias_sb[:], bias.rearrange("(nt p) -> p nt", p=P))

        xT3 = xT.rearrange("(c p) m -> p c m", p=P)  # [128, 32, 1024]
        wsT3 = wscanT.rearrange("(c p) m -> p c m", p=P)  # [128, 32, 512]
        wT3 = wT.rearrange("(kt p) n -> p kt n", p=P)  # [128, 32, 4096]

        # ---- 1. absmax scans (ring allocs 0..23: run at full HBM BW) ----
        wmax_cols = stats.tile([P, 8], F32)
        for i in range(8):
            tw = stage.tile([P, 4, WS], F32, tag="stg", name=f"wsc{i}")
            nc.sync.dma_start(tw[:], wsT3[:, i * 4 : (i + 1) * 4, :])
            nc.vector.tensor_reduce(
                wmax_cols[:, i : i + 1], tw[:], axis=AX.XY, op=ALU.max,
                apply_absolute_value=True,
            )
        # w collective #1: triggered as soon as the 8 MiB w scan is done
        # (~40us), long before the startup barrier even clears.  It pays the
        # slow first-collective latency (inter-core launch skew + CC spin-up)
        # entirely inside the x-scan window.
        lmax2 = stats.tile([P, 2], F32)
        nc.vector.tensor_reduce(lmax2[:, 0:1], wmax_cols[:], axis=AX.X, op=ALU.max)
        gmaxw = stats.tile([P, 1], F32)
        nc.gpsimd.partition_all_reduce(
            gmaxw[:], lmax2[:, 0:1], channels=P,
            reduce_op=bass_isa.ReduceOp.max,
        )
        cc_in_w = dram.tile([1, 1], F32)
        cc_out_w = dram.tile([1, 1], F32)
        # Tiny collective DMAs ride the otherwise-idle Activation HWDGE ring
        # so they don't queue behind in-flight multi-MiB prefetch packets.
        nc.scalar.dma_start(cc_in_w[:], gmaxw[0:1, :])
        nc.gpsimd.collective_compute(
            "AllReduce", ALU.max, replica_groups=[list(range(n_cores))],
            ins=[cc_in_w.opt()], outs=[cc_out_w.opt()],
        )

        xmax_cols = stats.tile([P, 16], F32)
        for i in range(16):
            tx = stage.tile([P, 2, m_loc], F32, tag="stg", name=f"xsc{i}")
            nc.sync.dma_start(tx[:], xT3[:, i * 2 : (i + 1) * 2, :])
            nc.vector.tensor_reduce(
                xmax_cols[:, i : i + 1], tx[:], axis=AX.XY, op=ALU.max,
                apply_absolute_value=True,
            )
        nc.vector.tensor_reduce(lmax2[:, 1:2], xmax_cols[:], axis=AX.X, op=ALU.max)

        # x collective #2: triggered the moment the x scan lands.  The w
        # collective (#1, triggered ~50us earlier, below) absorbs the
        # inter-core NEFF-launch skew, so this one runs at the fast
        # all-cores-aligned latency right after it on the CC stream.
        gmax2 = stats.tile([P, 2], F32)
        nc.gpsimd.partition_all_reduce(
            gmax2[:, 1:2], lmax2[:, 1:2], channels=P,
            reduce_op=bass_isa.ReduceOp.max,
        )
        cc_in_x = dram.tile([1, 1], F32)
        cc_out_x = dram.tile([1, 1], F32)
        nc.scalar.dma_start(cc_in_x[:], gmax2[0:1, 1:2])
        nc.gpsimd.collective_compute(
            "AllReduce", ALU.max, replica_groups=[list(range(n_cores))],
            ins=[cc_in_x.opt()], outs=[cc_out_x.opt()],
        )

        # ---- 3. gated prefetch: ring WAR defers these behind the scans --
        # (DMA triggers only; consumed post-collective.)
        wf_tiles = {}  # (s, half) -> f32 stage tile [P, 16, 128]
        xre_tiles = {}  # (mh, ck) -> f32 stage tile [P, 4, 512]

        def load_wf(s):
            for h in range(2):
                t = stage.tile([P, 16, NSTRIP], F32, tag="stg", name=f"wf{s}_{h}")
                nc.sync.dma_start(
                    t[:],
                    wT3[:, h * 16 : (h + 1) * 16,
                        s * NSTRIP : (s + 1) * NSTRIP],
                )
                wf_tiles[(s, h)] = t

        def load_xre(mh, ck):
            t = stage.tile([P, 4, MFREE], F32, tag="stg", name=f"xr{mh}_{ck}")
            nc.sync.dma_start(
                t[:],
                xT3[:, ck * 4 : (ck + 1) * 4,
                    mh * MFREE : (mh + 1) * MFREE],
            )
            xre_tiles[(mh, ck)] = t

        load_wf(0)
        load_wf(1)
        for ck in range(n_ck):
            load_xre(0, ck)
        load_wf(2)
        load_wf(3)
        for ck in range(n_ck):
            load_xre(1, ck)

        # ---- 4a. w scales (available during the x collective) -----------
        gsbw = stats.tile([1, 1], F32)
        nc.scalar.dma_start(gsbw[:], cc_out_w[:])
        wrec = stats.tile([1, 1], F32)
        s_w = stats.tile([1, 1], F32)
        sw2 = stats.tile([1, 2], F32)
        nc.vector.reciprocal(wrec[:], gsbw[:])
        nc.vector.tensor_scalar(sw2[:, 0:1], wrec[:], 127.0, None, op0=ALU.mult)
        nc.vector.tensor_scalar(s_w[:], gsbw[:], INV127, None, op0=ALU.mult)
        scbw = const.tile([P, 1], F32)
        nc.gpsimd.partition_broadcast(scbw[:], sw2[:, 0:1])
        inv_sw = scbw[:, 0:1]

        # ---- 5. quantize helpers ----------------------------------------
        wq_tiles = {}

        def quant_w_strip(s, on_act=True):
            wq = wq_pool.tile([P, kt_n, NSTRIP], F16, tag="wq", name=f"wq{s}")
            for h in range(2):
                sl = wq[:, h * 16 : (h + 1) * 16, :]
                src = wf_tiles.pop((s, h))[:]
                if on_act:
                    nc.scalar.activation(
                        sl, src, ACTF.Copy, bias=MAGIC, scale=inv_sw
                    )
                else:
                    nc.vector.tensor_scalar(
                        sl, src, inv_sw, MAGIC, op0=ALU.mult, op1=ALU.add
                    )
                nc.vector.tensor_scalar(sl, sl, MAGIC, None, op0=ALU.subtract)
            wq_tiles[s] = wq

        # ---- 5b. w strips 0-3 quantize inside the x-collective window ---
        for s in range(4):
            quant_w_strip(s)

        # ---- 4b. x scales (the only post-x-collective serial work) ------
        gsbx = stats.tile([1, 1], F32)
        nc.scalar.dma_start(gsbx[:], cc_out_x[:])
        xrec = stats.tile([1, 1], F32)
        s_x = stats.tile([1, 1], F32)
        sx2 = stats.tile([1, 2], F32)
        nc.vector.reciprocal(xrec[:], gsbx[:])
        nc.vector.tensor_scalar(sx2[:, 0:1], xrec[:], 127.0, None, op0=ALU.mult)
        nc.vector.tensor_scalar(s_x[:], gsbx[:], INV127, None, op0=ALU.mult)
        nc.vector.tensor_tensor(sx2[:, 1:2], s_w[:], s_x[:], op=ALU.mult)
        scbx = const.tile([P, 2], F32)
        nc.gpsimd.partition_broadcast(scbx[:], sx2[:])
        inv_sx = scbx[:, 0:1]
        out_sc = scbx[:, 1:2]

        xqs = [
            xq_pool.tile([P, kt_n, MFREE], F16, tag=f"xq{h}", name=f"xq{h}")
            for h in range(n_mh)
        ]

        def quant_x_chunk(mh, ck, on_act):
            sl = xqs[mh][:, ck * 4 : (ck + 1) * 4, :]
            src = xre_tiles.pop((mh, ck))[:]
            if on_act:
                nc.scalar.activation(sl, src, ACTF.Copy, bias=MAGIC, scale=inv_sx)
            else:
                nc.vector.tensor_scalar(
                    sl, src, inv_sx, MAGIC, op0=ALU.mult, op1=ALU.add
                )
            nc.vector.tensor_scalar(sl, sl, MAGIC, None, op0=ALU.subtract)

        # ---- 6. x quantize (w is already done; DVE+ACT both free) -------
        quant_x_chunk(0, 0, on_act=False)
        for ck in range(1, n_ck):
            quant_x_chunk(0, ck, on_act=(ck % 2 == 0))
        for ck in range(n_ck):
            quant_x_chunk(1, ck, on_act=(ck % 2 == 0))

        # ---- 7. stream: per strip s: prefetch + quantize s+4, MM s ------
        for s in range(n_strips):
            if s + 4 < n_strips:
                load_wf(s + 4)
                quant_w_strip(s + 4)
            wq = wq_tiles.pop(s)
            for mh in range(n_mh):
                ps = ps_pool.tile([P, MFREE], F32)
                for kt in range(kt_n):
                    nc.tensor.matmul(
                        ps[:],
                        wq[:, kt, :],
                        xqs[mh][:, kt, :],
                        start=(kt == 0),
                        stop=(kt == kt_n - 1),
                    )
                ob = ob_pool.tile([P, MFREE], F32, tag="ob")
                nc.vector.tensor_scalar(
                    ob[:], ps[:], out_sc, bias_sb[:, s : s + 1],
                    op0=ALU.mult, op1=ALU.add,
                )
                nc.gpsimd.dma_start(
                    outT[s * NSTRIP : (s + 1) * NSTRIP,
                         mh * MFREE : (mh + 1) * MFREE],
                    ob[:],
                )


def build_nc(m_loc=MLOC, k=K, n=N, ws=WS, n_cores=NCORES):
    nc = bacc.Bacc("TRN2", target_bir_lowering=False, debug=False,
                   num_devices=n_cores)
    xT = nc.dram_tensor("xT", [k, m_loc], F32, kind="ExternalInput").ap()
    wT = nc.dram_tensor("wT", [k, n], F32, kind="ExternalInput").ap()
    wscanT = nc.dram_tensor("wscanT", [k, ws], F32, kind="ExternalInput").ap()
    bias = nc.dram_tensor("bias", [n], F32, kind="ExternalInput").ap()
    outT = nc.dram_tensor("outT", [n, m_loc], F32, kind="ExternalOutput").ap()
    with tile.TileContext(nc) as tc:
        build_body(tc, xT, wT, wscanT, bias, outT, n_cores=n_cores)
    nc.compile()
    return nc


def make_in_maps(x, weight, bias, n_cores=NCORES):
    m_loc = x.shape[0] // n_cores
    ws = weight.shape[0] // n_cores
    wT = np.ascontiguousarray(weight.T)
    bias = np.ascontiguousarray(bias, dtype=np.float32)
    maps = []
    for c in range(n_cores):
        maps.append({
            "xT": np.ascontiguousarray(x[c * m_loc : (c + 1) * m_loc].T),
            "wT": wT,
            "wscanT": np.ascontiguousarray(weight[c * ws : (c + 1) * ws].T),
            "bias": bias,
        })
    return maps


_NC_CACHE = {}
LAST_RUN = None


def kernel(x, weight, bias, _trace=False):
    global LAST_RUN
    x = np.ascontiguousarray(np.asarray(x), dtype=np.float32)
    weight = np.ascontiguousarray(np.asarray(weight), dtype=np.float32)
    bias = np.asarray(bias, dtype=np.float32)
    if "full" not in _NC_CACHE:
        _NC_CACHE["full"] = build_nc()
    nc = _NC_CACHE["full"]
    in_maps = make_in_maps(x, weight, bias)
    res = bass_utils.run_bass_kernel_spmd(
        nc, in_maps, core_ids=list(range(NCORES)), trace=_trace
    )
    LAST_RUN = res
    out = np.empty((M, N), np.float32)
    for c in range(NCORES):
        out[c * MLOC : (c + 1) * MLOC, :] = res.results[c]["outT"].T
    return out


# revision 12
# speedup vs baseline: 1.0353x; 1.0350x over previous
"""Quantized linear (dynamic per-tensor int8) on 8 TRN2 NeuronCores.

Reference semantics:
    x_q = round(x / s_x), s_x = max|x|/127   (per-tensor, round-half-even)
    w_q = round(w / s_w), s_w = max|w|/127
    out = (x_q @ w_q.T) * (s_x * s_w) + bias

Distribution: data-parallel over M (8 shards of 1024 rows), weight
replicated.  Each core scans a disjoint 1/8 of x (its own shard) and of w
for the local absmax; a single 2-element AllReduce(max) collective produces
both global scales in one round trip.  Quantized values are exact small
integers held in fp16 (ints <= 2047 are exact in fp16), so the TensorE fp16
matmul with fp32 PSUM accumulation reproduces the int8 GEMM exactly (all
partial sums stay far below 2^24).

Rounding uses an fp16 magic: (v*inv_s + 1536) written to fp16 rounds the
fractional part half-to-even (ulp = 1 in [1024, 2048)), then an in-place
fp16 subtract of 1536 (2x DVE rate) recovers the integer, matching
jnp.round.

Scheduling notes (engine FIFOs execute in emission order):
  * All large f32 staging goes through ONE pool tag ("stg", 10 x 1 MiB
    ring).  The ring's WAR chain defers every re-read/prefetch DMA until
    the absmax-scan chunks it displaces have been consumed, so the scan
    (which gates the collective) gets full HBM bandwidth first, and the
    re-reads then land inside the collective's latency window.
  * One fused AllReduce instead of two staggered ones removes a full
    collective round trip plus the strict-FIFO GpSimd serialization
    between them.
  * The w-strip quantize for strip s+2 is emitted BEFORE the PSUM
    evacuations of strip s so the DVE FIFO never makes the TensorE wait.
"""

import numpy as np

from concourse import bacc, bass_isa
import concourse.bass_utils as bass_utils
import concourse.mybir as mybir
import concourse.tile as tile

P = 128
M, K, N = 8192, 4096, 4096
NCORES = 8
MLOC = M // NCORES  # 1024 rows of x per core
WS = N // NCORES  # 512 columns of wT scanned per core for absmax
MAGIC = 1536.0  # fp16 round-to-int magic: [1024,2048) has ulp 1
MFREE = 512  # moving free dim per matmul (one fp32 PSUM bank)
NSTRIP = 128  # n-columns of w quantized per strip
INV127 = float(np.float32(1.0 / 127.0))

F32 = mybir.dt.float32
F16 = mybir.dt.float16
AX = mybir.AxisListType
ALU = mybir.AluOpType
ACTF = mybir.ActivationFunctionType


def build_body(tc, xT, wT, wscanT, bias, outT, *, n_cores):
    nc = tc.nc
    k, m_loc = xT.shape
    n = wT.shape[1]
    kt_n = k // P  # 32
    n_strips = n // NSTRIP  # 32
    n_mh = m_loc // MFREE  # 2
    n_ck = kt_n // 4  # 8 quantize chunks of 4 k-tiles per mh

    with (
        tc.tile_pool(name="const", bufs=1) as const,
        tc.tile_pool(name="stats", bufs=1) as stats,
        tc.tile_pool(name="stage", bufs=6) as stage,
        tc.tile_pool(name="xq", bufs=1) as xq_pool,
        tc.tile_pool(name="wq", bufs=4) as wq_pool,
        tc.tile_pool(name="ob", bufs=4) as ob_pool,
        tc.tile_pool(name="ps", bufs=6, space="PSUM") as ps_pool,
        tc.tile_pool(name="dram", bufs=1, space="DRAM") as dram,
    ):
        # ---- bias, laid out bias[s*128+p] -> bias_sb[p, s] ---------------
        bias_sb = const.tile([P, n // P], F32)
        nc.sync.dma_start(bias_sb[:], bias.rearrange("(nt p) -> p nt", p=P))

        xT3 = xT.rearrange("(c p) m -> p c m", p=P)  # [128, 32, 1024]
        wsT3 = wscanT.rearrange("(c p) m -> p c m", p=P)  # [128, 32, 512]
        wT3 = wT.rearrange("(kt p) n -> p kt n", p=P)  # [128, 32, 4096]

        # ---- 1. absmax scans (ring allocs 0..23: run at full HBM BW) ----
        wmax_cols = stats.tile([P, 8], F32)
        for i in range(8):
            tw = stage.tile([P, 4, WS], F32, tag="stg", name=f"wsc{i}")
            nc.sync.dma_start(tw[:], wsT3[:, i * 4 : (i + 1) * 4, :])
            nc.vector.tensor_reduce(
                wmax_cols[:, i : i + 1], tw[:], axis=AX.XY, op=ALU.max,
                apply_absolute_value=True,
            )
        # w collective #1: triggered as soon as the 8 MiB w scan is done
        # (~40us), long before the startup barrier even clears.  It pays the
        # slow first-collective latency (inter-core launch skew + CC spin-up)
        # entirely inside the x-scan window.
        lmax2 = stats.tile([P, 2], F32)
        nc.vector.tensor_reduce(lmax2[:, 0:1], wmax_cols[:], axis=AX.X, op=ALU.max)
        gmaxw = stats.tile([P, 1], F32)
        nc.gpsimd.partition_all_reduce(
            gmaxw[:], lmax2[:, 0:1], channels=P,
            reduce_op=bass_isa.ReduceOp.max,
        )
        cc_in_w = dram.tile([1, 1], F32)
        cc_out_w = dram.tile([1, 1], F32)
        # Tiny collective DMAs ride the otherwise-idle Activation HWDGE ring
        # so they don't queue behind in-flight multi-MiB prefetch packets.
        nc.scalar.dma_start(cc_in_w[:], gmaxw[0:1, :])
        nc.gpsimd.collective_compute(
            "AllReduce", ALU.max, replica_groups=[list(range(n_cores))],
            ins=[cc_in_w.opt()], outs=[cc_out_w.opt()],
        )

        xmax_cols = stats.tile([P, 16], F32)
        for i in range(16):
            tx = stage.tile([P, 2, m_loc], F32, tag="stg", name=f"xsc{i}")
            nc.sync.dma_start(tx[:], xT3[:, i * 2 : (i + 1) * 2, :])
            nc.vector.tensor_reduce(
                xmax_cols[:, i : i + 1], tx[:], axis=AX.XY, op=ALU.max,
                apply_absolute_value=True,
            )
        nc.vector.tensor_reduce(lmax2[:, 1:2], xmax_cols[:], axis=AX.X, op=ALU.max)

        # x collective #2: triggered the moment the x scan lands.  The w
        # collective (#1, triggered ~50us earlier, below) absorbs the
        # inter-core NEFF-launch skew, so this one runs at the fast
        # all-cores-aligned latency right after it on the CC stream.
        gmax2 = stats.tile([P, 2], F32)
        nc.gpsimd.partition_all_reduce(
            gmax2[:, 1:2], lmax2[:, 1:2], channels=P,
            reduce_op=bass_isa.ReduceOp.max,
        )
        cc_in_x = dram.tile([1, 1], F32)
        cc_out_x = dram.tile([1, 1], F32)
        nc.scalar.dma_start(cc_in_x[:], gmax2[0:1, 1:2])
        nc.gpsimd.collective_compute(
            "AllReduce", ALU.max, replica_groups=[list(range(n_cores))],
            ins=[cc_in_x.opt()], outs=[cc_out_x.opt()],
        )

        # ---- 3. gated prefetch: ring WAR defers these behind the scans --
        # (DMA triggers only; consumed post-collective.)
        wf_tiles = {}  # (s, half) -> f32 stage tile [P, 16, 128]
        xre_tiles = {}  # (mh, ck) -> f32 stage tile [P, 4, 512]

        def load_wf(s):
            for h in range(2):
                t = stage.tile([P, 16, NSTRIP], F32, tag="stg", name=f"wf{s}_{h}")
                nc.sync.dma_start(
                    t[:],
                    wT3[:, h * 16 : (h + 1) * 16,
                        s * NSTRIP : (s + 1) * NSTRIP],
                )
                wf_tiles[(s, h)] = t

        def load_xre(mh, ck):
            t = stage.tile([P, 4, MFREE], F32, tag="stg", name=f"xr{mh}_{ck}")
            nc.sync.dma_start(
                t[:],
                xT3[:, ck * 4 : (ck + 1) * 4,
                    mh * MFREE : (mh + 1) * MFREE],
            )
            xre_tiles[(mh, ck)] = t

        load_wf(0)
        load_wf(1)
        for ck in range(n_ck):
            load_xre(0, ck)
        load_wf(2)
        load_wf(3)
        for ck in range(n_ck):
            load_xre(1, ck)

        # ---- 4a. w scales (available during the x collective) -----------
        gsbw = stats.tile([1, 1], F32)
        nc.scalar.dma_start(gsbw[:], cc_out_w[:])
        wrec = stats.tile([1, 1], F32)
        s_w = stats.tile([1, 1], F32)
        sw2 = stats.tile([1, 2], F32)
        nc.vector.reciprocal(wrec[:], gsbw[:])
        nc.vector.tensor_scalar(sw2[:, 0:1], wrec[:], 127.0, None, op0=ALU.mult)
        nc.vector.tensor_scalar(s_w[:], gsbw[:], INV127, None, op0=ALU.mult)
        scbw = const.tile([P, 1], F32)
        nc.gpsimd.partition_broadcast(scbw[:], sw2[:, 0:1])
        inv_sw = scbw[:, 0:1]

        # ---- 5. quantize helpers ----------------------------------------
        wq_tiles = {}

        def quant_w_strip(s, on_act=True):
            wq = wq_pool.tile([P, kt_n, NSTRIP], F16, tag="wq", name=f"wq{s}")
            for h in range(2):
                sl = wq[:, h * 16 : (h + 1) * 16, :]
                src = wf_tiles.pop((s, h))[:]
                if on_act:
                    nc.scalar.activation(
                        sl, src, ACTF.Copy, bias=MAGIC, scale=inv_sw
                    )
                else:
                    nc.vector.tensor_scalar(
                        sl, src, inv_sw, MAGIC, op0=ALU.mult, op1=ALU.add
                    )
                nc.vector.tensor_scalar(sl, sl, MAGIC, None, op0=ALU.subtract)
            wq_tiles[s] = wq

        # ---- 5b. w strips 0-3 quantize inside the x-collective window ---
        for s in range(4):
            quant_w_strip(s)

        # ---- 4b. x scales (the only post-x-collective serial work) ------
        gsbx = stats.tile([1, 1], F32)
        nc.scalar.dma_start(gsbx[:], cc_out_x[:])
        xrec = stats.tile([1, 1], F32)
        s_x = stats.tile([1, 1], F32)
        sx2 = stats.tile([1, 2], F32)
        nc.vector.reciprocal(xrec[:], gsbx[:])
        nc.vector.tensor_scalar(sx2[:, 0:1], xrec[:], 127.0, None, op0=ALU.mult)
        nc.vector.tensor_scalar(s_x[:], gsbx[:], INV127, None, op0=ALU.mult)
        nc.vector.tensor_tensor(sx2[:, 1:2], s_w[:], s_x[:], op=ALU.mult)
        scbx = const.tile([P, 2], F32)
        nc.gpsimd.partition_broadcast(scbx[:], sx2[:])
        inv_sx = scbx[:, 0:1]
        out_sc = scbx[:, 1:2]

        xqs = [
            xq_pool.tile([P, kt_n, MFREE], F16, tag=f"xq{h}", name=f"xq{h}")
            for h in range(n_mh)
        ]

        def quant_x_chunk(mh, ck, on_act):
            sl = xqs[mh][:, ck * 4 : (ck + 1) * 4, :]
            src = xre_tiles.pop((mh, ck))[:]
            if on_act:
                nc.scalar.activation(sl, src, ACTF.Copy, bias=MAGIC, scale=inv_sx)
            else:
                nc.vector.tensor_scalar(
                    sl, src, inv_sx, MAGIC, op0=ALU.mult, op1=ALU.add
                )
            nc.vector.tensor_scalar(sl, sl, MAGIC, None, op0=ALU.subtract)

        # ---- 6. x quantize (w is already done; DVE+ACT both free) -------
        quant_x_chunk(0, 0, on_act=False)
        for ck in range(1, n_ck):
            quant_x_chunk(0, ck, on_act=(ck % 2 == 0))
        for ck in range(n_ck):
            quant_x_chunk(1, ck, on_act=(ck % 2 == 0))

        # ---- 7. stream: per strip s: prefetch + quantize s+4, MM s ------
        for s in range(n_strips):
            if s + 4 < n_strips:
                load_wf(s + 4)
                quant_w_strip(s + 4)
            wq = wq_tiles.pop(s)
            for mh in range(n_mh):
                ps = ps_pool.tile([P, MFREE], F32)
                for kt in range(kt_n):
                    nc.tensor.matmul(
                        ps[:],
                        wq[:, kt, :],
                        xqs[mh][:, kt, :],
                        start=(kt == 0),
                        stop=(kt == kt_n - 1),
                    )
                ob = ob_pool.tile([P, MFREE], F32, tag="ob")
                nc.vector.tensor_scalar(
                    ob[:], ps[:], out_sc, bias_sb[:, s : s + 1],
                    op0=ALU.mult, op1=ALU.add,
                )
                nc.gpsimd.dma_start(
                    outT[s * NSTRIP : (s + 1) * NSTRIP,
                         mh * MFREE : (mh + 1) * MFREE],
                    ob[:],
                )


def build_nc(m_loc=MLOC, k=K, n=N, ws=WS, n_cores=NCORES):
    nc = bacc.Bacc("TRN2", target_bir_lowering=False, debug=False,
                   num_devices=n_cores)
    xT = nc.dram_tensor("xT", [k, m_loc], F32, kind="ExternalInput").ap()
    wT = nc.dram_tensor("wT", [k, n], F32, kind="ExternalInput").ap()
    wscanT = nc.dram_tensor("wscanT", [k, ws], F32, kind="ExternalInput").ap()
    bias = nc.dram_tensor("bias", [n], F32, kind="ExternalInput").ap()
    outT = nc.dram_tensor("outT", [n, m_loc], F32, kind="ExternalOutput").ap()
    with tile.TileContext(nc) as tc:
        build_body(tc, xT, wT, wscanT, bias, outT, n_cores=n_cores)
    nc.compile()
    return nc


def make_in_maps(x, weight, bias, n_cores=NCORES):
    m_loc = x.shape[0] // n_cores
    ws = weight.shape[0] // n_cores
    wT = np.ascontiguousarray(weight.T)
    bias = np.ascontiguousarray(bias, dtype=np.float32)
    maps = []
    for c in range(n_cores):
        maps.append({
            "xT": np.ascontiguousarray(x[c * m_loc : (c + 1) * m_loc].T),
            "wT": wT,
            "wscanT": np.ascontiguousarray(weight[c * ws : (c + 1) * ws].T),
            "bias": bias,
        })
    return maps


_NC_CACHE = {}
LAST_RUN = None


def kernel(x, weight, bias, _trace=False):
    global LAST_RUN
    x = np.ascontiguousarray(np.asarray(x), dtype=np.float32)
    weight = np.ascontiguousarray(np.asarray(weight), dtype=np.float32)
    bias = np.asarray(bias, dtype=np.float32)
    if "full" not in _NC_CACHE:
        _NC_CACHE["full"] = build_nc()
    nc = _NC_CACHE["full"]
    in_maps = make_in_maps(x, weight, bias)
    res = bass_utils.run_bass_kernel_spmd(
        nc, in_maps, core_ids=list(range(NCORES)), trace=_trace
    )
    LAST_RUN = res
    out = np.empty((M, N), np.float32)
    for c in range(NCORES):
        out[c * MLOC : (c + 1) * MLOC, :] = res.results[c]["outT"].T
    return out
